# revision 51
# baseline (speedup 1.0000x reference)
"""GQA attention (BagleyAttention) on 8 Trainium2 NeuronCores.

Tensor-parallel over kv-head groups: core c owns kv head c and query heads
[4c, 4c+4). Each core computes its heads' attention and a partial output
projection [S, D]; the host sums the 8 partials.

v3: software-pipelined rounds — round r runs the sb=r QKV projection on the
PE while the previous round's attention (t=r-1) and the round-before's
output projection (t=r-2) fill the PE between exp-paced score chunks.
Softmax denominators accumulate in fp16 on DVE/GpSimd; causal diagonal
chunks stream only the valid q columns; rotate_half is a signed
permutation matmul (no SBUF-SBUF DMA); all HBM traffic uses host-pretiled
layouts so each DMA descriptor moves >=4KB per partition (the DMA fabric
is packet-rate-bound at ~1KB granularity).
"""

import math
import sys

sys.path.insert(0, "/opt/trn_rl_repo")

import numpy as np

# Problem sizes (hardcoded per contract; kernel.py reads no sibling files).
B, S, D = 1, 2048, 4096
H, KV, Dh = 32, 8, 128
G = H // KV            # query heads per kv head (= per core)
EH = G * Dh            # per-core q projection width (512)
N_CORES = 8

SB = 512               # s-block width (projection s-block = attention q-block)
N_SB = S // SB         # 4 rounds
N_DC = D // 128        # 32 d-chunks
N_DCG = 8              # d-chunk groups of 4 (x staging granularity)
N_NB = D // SB         # 8 output d-blocks

EXP_BIAS = 9.5         # exp(s - EXP_BIAS); cancels in softmax normalization

DEBUG_TAPS = False     # add qr/kr/vnat/attn dram dumps (debugging only)

_cache = {}


def _build():
    import concourse.bass as bass
    import concourse.mybir as mybir
    import concourse.tile as tile
    from concourse import bacc
    from concourse.masks import make_identity

    dt = mybir.dt
    f32, f16 = dt.float32, dt.float16
    AF = mybir.ActivationFunctionType

    nc = bacc.Bacc("TRN2", target_bir_lowering=False, debug=False)

    # host-pretiled inputs (see _prep_inputs for layouts)
    xg = nc.dram_tensor("xg", [N_SB, N_DCG, 128, 4, SB], f16,
                        kind="ExternalInput").ap()
    wqt = nc.dram_tensor("wqt", [128, N_DC, EH], f16,
                         kind="ExternalInput").ap()
    wkt = nc.dram_tensor("wkt", [128, N_DC, Dh], f16,
                         kind="ExternalInput").ap()
    wvt = nc.dram_tensor("wvt", [128, N_DC, Dh], f16,
                         kind="ExternalInput").ap()
    wot = nc.dram_tensor("wot", [128, G, D], f16, kind="ExternalInput").ap()
    cosT = nc.dram_tensor("cosT", [Dh, S], f16, kind="ExternalInput").ap()
    sinT = nc.dram_tensor("sinT", [Dh, S], f16, kind="ExternalInput").ap()
    triT = nc.dram_tensor("triT", [128, 128], f16, kind="ExternalInput").ap()
    permT = nc.dram_tensor("permT", [128, 128], f16,
                           kind="ExternalInput").ap()
    out = nc.dram_tensor("out", [S, D], f16, kind="ExternalOutput").ap()
    if DEBUG_TAPS:
        dbg_qr = nc.dram_tensor("dbg_qr", [5, N_SB, 128, SB], f16,
                                kind="ExternalOutput").ap()
        dbg_vn = nc.dram_tensor("dbg_vn", [N_SB, 128, N_SB * Dh], f16,
                                kind="ExternalOutput").ap()
        dbg_at = nc.dram_tensor("dbg_at", [G, N_SB, 128, SB], f16,
                                kind="ExternalOutput").ap()

    inv_sqrt_dh = 1.0 / math.sqrt(Dh)

    with tile.TileContext(nc) as tc, \
         tc.tile_pool(name="persist", bufs=1) as persist, \
         tc.tile_pool(name="mixp", bufs=6, space="PSUM") as mixp, \
         tc.tile_pool(name="scp", bufs=2, space="PSUM") as scp, \
         tc.tile_pool(name="xstage", bufs=9) as xstage, \
         tc.tile_pool(name="ropeb", bufs=1) as ropeb, \
         tc.tile_pool(name="expp", bufs=6) as expp, \
         tc.tile_pool(name="eaccp", bufs=2) as eaccp, \
         tc.tile_pool(name="miscp", bufs=2) as miscp, \
         tc.tile_pool(name="obuf", bufs=2) as obuf:

        # ---- long-lived SBUF tensors --------------------------------------
        qr = [[persist.tile([128, SB], f16, tag=f"qr{h}_{sb}",
                            name=f"qr{h}_{sb}") for sb in range(N_SB)]
              for h in range(G)]
        kr = [persist.tile([128, SB], f16, tag=f"kr{sb}", name=f"kr{sb}")
              for sb in range(N_SB)]
        vnat = [persist.tile([128, N_SB * Dh], f16, tag=f"vn{sb}",
                             name=f"vn{sb}") for sb in range(N_SB)]
        attn = [[persist.tile([128, SB], f16, tag=f"attn{h}_{t}",
                              name=f"attn{h}_{t}") for t in range(N_SB)]
                for h in range(G)]

        wq_h = persist.tile([128, N_DC, EH], f16, tag="wq_h", name="wq_h")
        wk_h = persist.tile([128, N_DC, Dh], f16, tag="wk_h", name="wk_h")
        wv_h = persist.tile([128, N_DC, Dh], f16, tag="wv_h", name="wv_h")
        wo_r = persist.tile([128, G, D], f16, tag="wo_r", name="wo_r")
        cos_sb = persist.tile([128, S], f16, tag="cos", name="cos_sb")
        sin_sb = persist.tile([128, S], f16, tag="sin", name="sin_sb")
        tri = persist.tile([128, 128], f16, tag="tri", name="tri")
        perm = persist.tile([128, 128], f16, tag="perm", name="perm")
        ones_h = persist.tile([128, 128], f16, tag="ones", name="ones_h")
        ident = persist.tile([128, 128], f16, tag="ident", name="ident")
        ebias = persist.tile([128, 1], f32, tag="ebias", name="ebias")

        nc.vector.memset(ones_h, 1.0)
        nc.vector.memset(ebias, -EXP_BIAS)
        make_identity(nc, ident)

        # PE warm-up: no-dep matmuls run during the initial DMA wait,
        # keeping the HAM activity window busy so the first projection
        # matmuls start at the full 2.4 GHz clock instead of 1.2 GHz.
        wsrc = persist.tile([128, SB], f16, tag="wsrc", name="wsrc")
        nc.vector.memset(wsrc, 0.000244140625)
        warm = mixp.tile([128, SB], f32, tag="mix", name="warm")
        for _ in range(28):
            nc.tensor.matmul(warm[:], ones_h[:], wsrc[:],
                             start=True, stop=True)

        # RoPE scratch (per projection head slot: 4 q + 1 k)
        t_plain = [ropeb.tile([128, SB], f16, tag=f"plain{i}",
                              name=f"plain{i}") for i in range(5)]
        t_cos = [ropeb.tile([128, SB], f16, tag=f"tcos{i}",
                            name=f"tcos{i}") for i in range(5)]
        t_sw = [ropeb.tile([128, SB], f16, tag=f"tsw{i}",
                           name=f"tsw{i}") for i in range(5)]
        vt_sb = ropeb.tile([128, SB], f16, tag="vt", name="vt_sb")

        # -------------------------------------------------------------------
        # DMA emission (all host-pretiled, big per-partition rows)
        # -------------------------------------------------------------------
        xq_tiles = {}   # (sb, dcg) -> tile [128, 4, SB]

        def emit_x_dma(sb):
            for dcg in range(N_DCG):
                xq = xstage.tile([128, 4, SB], f16, tag="xq",
                                 name=f"xq{sb}_{dcg}")
                nc.sync.dma_start(out=xq, in_=xg[sb, dcg])
                xq_tiles[(sb, dcg)] = xq

        def emit_weight_dma():
            # order: first dc-group of everything, then interleave so DMA
            # supply stays ~2 dc-groups ahead of the PE during round 0.
            def wq_g(g):
                cs = slice(g * 8, (g + 1) * 8)
                nc.sync.dma_start(out=wq_h[:, cs, :], in_=wqt[:, cs, :])

            def wkv_g(g):
                cs = slice(g * 16, (g + 1) * 16)
                nc.sync.dma_start(out=wk_h[:, cs, :], in_=wkt[:, cs, :])
                nc.sync.dma_start(out=wv_h[:, cs, :], in_=wvt[:, cs, :])

            def xq_g(dcg):
                xq = xstage.tile([128, 4, SB], f16, tag="xq",
                                 name=f"xq0_{dcg}")
                nc.sync.dma_start(out=xq, in_=xg[0, dcg])
                xq_tiles[(0, dcg)] = xq

            # small first descriptors so the first matmul starts ASAP
            nc.sync.dma_start(out=wq_h[:, 0:2, :], in_=wqt[:, 0:2, :])
            nc.sync.dma_start(out=wk_h[:, 0:4, :], in_=wkt[:, 0:4, :])
            nc.sync.dma_start(out=wv_h[:, 0:4, :], in_=wvt[:, 0:4, :])
            xq_g(0)
            nc.sync.dma_start(out=wq_h[:, 2:8, :], in_=wqt[:, 2:8, :])
            nc.sync.dma_start(out=wk_h[:, 4:8, :], in_=wkt[:, 4:8, :])
            nc.sync.dma_start(out=wv_h[:, 4:8, :], in_=wvt[:, 4:8, :])
            xq_g(1)
            nc.sync.dma_start(out=wk_h[:, 8:16, :], in_=wkt[:, 8:16, :])
            nc.sync.dma_start(out=wv_h[:, 8:16, :], in_=wvt[:, 8:16, :])
            wq_g(1); xq_g(2); xq_g(3)
            wq_g(2); wkv_g(1); xq_g(4); xq_g(5)
            wq_g(3); xq_g(6); xq_g(7)
            # trig/masks land before their first readers (rope drain 0 /
            # rope tail 0 / first diagonal chunk); must be EMITTED before
            # those readers too, else Tile orders the load after them.
            nc.sync.dma_start(out=cos_sb, in_=cosT)
            nc.sync.dma_start(out=sin_sb, in_=sinT)
            nc.sync.dma_start(out=tri, in_=triT)
            nc.sync.dma_start(out=perm, in_=permT)

        # -------------------------------------------------------------------
        # Projection: 6 accumulation groups (4 q heads, k, v) over 32 d-chunks
        # -------------------------------------------------------------------
        def emit_proj(r):
            acc = [mixp.tile([128, SB], f32, tag="mix", name=f"acc{r}_{i}")
                   for i in range(6)]
            for dcg in range(N_DCG):
                xf = xq_tiles.pop((r, dcg))
                for j in range(4):
                    dc = dcg * 4 + j
                    xs = xf[:, j, :]
                    st_flags = dict(start=(dc == 0), stop=(dc == N_DC - 1))
                    for h in range(G):
                        nc.tensor.matmul(
                            acc[h][:], wq_h[:, dc, h * 128:(h + 1) * 128],
                            xs, **st_flags)
                    nc.tensor.matmul(acc[4][:], wk_h[:, dc, :], xs,
                                     **st_flags)
                    nc.tensor.matmul(acc[5][:], wv_h[:, dc, :], xs,
                                     **st_flags)
            return acc

        # RoPE part 1: drain the 6 PSUM banks (plain copies on ACT; cos-muls
        # on DVE). Frees "mix" banks for segment B; interleaved there.
        def make_drain_units(r, acc):
            ss = slice(r * SB, (r + 1) * SB)

            def drain(i):
                if i < 5:
                    # cos-mul reads the fp16 copy, not PSUM: 2x DVE rate and
                    # the accumulator bank frees after the single ACT read
                    nc.scalar.copy(out=t_plain[i], in_=acc[i][:])
                    nc.vector.tensor_mul(t_cos[i], t_plain[i], cos_sb[:, ss])
                else:
                    nc.scalar.copy(out=vt_sb, in_=acc[5][:])
            return [lambda i=i: drain(i) for i in range(6)]

        # RoPE part 2: rotate-half via signed permutation matmul, then
        # combine.  swp = P @ plain; dst = cos*acc + sin*swp.
        def make_tail_units(r):
            ss = slice(r * SB, (r + 1) * SB)

            def tail(i):
                swp = mixp.tile([128, SB], f32, tag="mix", name=f"swp{r}_{i}")
                nc.tensor.matmul(swp[:], perm[:], t_plain[i][:],
                                 start=True, stop=True)
                nc.vector.tensor_mul(t_sw[i], swp[:], sin_sb[:, ss])
                dst = qr[i][r] if i < G else kr[r]
                # final combine on GpSimd (SBUF-only op; keeps DVE free)
                nc.gpsimd.tensor_add(dst[:], t_cos[i], t_sw[i])
            return [lambda i=i: tail(i) for i in range(5)]

        # V natural layout: 4 PE transposes into one PSUM bank, one copy out.
        def emit_transposes(r):
            tp = mixp.tile([128, SB], f16, tag="mix", name=f"tp{r}")
            for j in range(4):
                nc.tensor.transpose(tp[:, j * 128:(j + 1) * 128],
                                    vt_sb[:, j * 128:(j + 1) * 128], ident[:])
            nc.vector.tensor_copy(vnat[r][:], tp[:])

        # -------------------------------------------------------------------
        # Attention for q-block t, head h (chunk-at-a-time emission)
        # -------------------------------------------------------------------
        att_state = {}

        def attn_begin(t, h):
            n = 4 * (t + 1)
            st = dict(n=n, pv=mixp.tile([128, SB], f32, tag="mix",
                                        name=f"pv{t}_{h}"))
            st["ed"] = eaccp.tile([128, SB], f16, tag="ed", name="ed")
            st["eg"] = eaccp.tile([128, SB], f16, tag="eg", name="eg")
            att_state[(t, h)] = st
            return st

        def attn_chunk(t, h, c):
            st = att_state[(t, h)]
            qlo = 128 * (c - 4 * t) if c >= 4 * t else 0
            W = SB - qlo
            sc = scp.tile([128, SB], f32, tag="sc", name="sc")
            nc.tensor.matmul(sc[:, 0:W], kr[c // 4][:, (c % 4) * 128:
                                                    (c % 4) * 128 + 128],
                             qr[h][t][:, qlo:SB], start=True, stop=True)
            e = expp.tile([128, SB], f16, tag="e", name="e")
            nc.scalar.activation(e[:, 0:W], sc[:, 0:W], AF.Exp,
                                 scale=inv_sqrt_dh, bias=ebias[:])
            # diagonal chunk: mask first 128 q-cols. On DVE (pv-matmul's
            # critical path) except t=0, where DVE is the binding engine
            # and the PE has slack to absorb GpSimd's latency.
            if c >= 4 * t:
                meng = nc.gpsimd if t == 0 else nc.vector
                meng.tensor_mul(e[:, 0:128], e[:, 0:128], tri[:])
            # E-accumulator: DVE owns even chunks plus the final (odd)
            # chunk — the Z-matmul should not wait on slower GpSimd adds.
            on_dve = (c % 2 == 0) or (c == st["n"] - 1)
            eng = nc.vector if on_dve else nc.gpsimd
            dst = st["ed"] if on_dve else st["eg"]
            if c == 1 and t == 0:
                # odd accumulator's first chunk only covers q >= 128
                nc.gpsimd.memset(st["eg"][:, 0:128], 0.0)
            if c <= 1:
                eng.tensor_copy(dst[:, qlo:SB], e[:, 0:W])
            else:
                eng.tensor_add(dst[:, qlo:SB], dst[:, qlo:SB], e[:, 0:W])
            vw = vnat[c // 4][:, (c % 4) * Dh:(c % 4 + 1) * Dh]
            nc.tensor.matmul(st["pv"][:, qlo:SB], vw, e[:, 0:W],
                             start=(c == 0), stop=(c == st["n"] - 1))

        def attn_eplg(t, h):
            st = att_state.pop((t, h))
            zb = mixp.tile([128, SB], f32, tag="mix", name=f"z{t}_{h}")
            nc.tensor.matmul(zb[:], ones_h[:], st["ed"][:],
                             start=True, stop=False)
            nc.tensor.matmul(zb[:], ones_h[:], st["eg"][:],
                             start=False, stop=True)
            rz = miscp.tile([128, SB], f32, tag="rz", name="rz")
            nc.vector.reciprocal_approx_fast(out=rz, in_=zb[:])
            nc.vector.tensor_mul(attn[h][t][:], st["pv"][:], rz[:])

        # -------------------------------------------------------------------
        # Output projection: 8 groups of 4 d-blocks per q-block; one DMA
        # per group ([128, 2048] = 4KB rows).
        # -------------------------------------------------------------------
        ob_state = {}

        def outproj_tile(t, i, ob_eng):
            sl = (i // N_NB) * 128          # s-offset within block
            nb = i % N_NB
            st_row = 4 * t + i // N_NB
            op = mixp.tile([128, SB], f32, tag="mix", name=f"op{t}_{i}")
            for hh in range(G):
                nc.tensor.matmul(op[:], attn[hh][t][:, sl:sl + 128],
                                 wo_r[:, hh, nb * SB:(nb + 1) * SB],
                                 start=(hh == 0), stop=(hh == G - 1))
            # group width: 4 d-blocks per DMA; 2 for the very last s-tile so
            # the final transfer starts earlier (shorter kernel tail)
            w = 2 if (t == 3 and i >= 24) else 4
            if nb % w == 0:
                ob_state[st_row] = obuf.tile([128, w * SB], f16, tag="ob",
                                             name=f"ob{t}_{i}")
            ob = ob_state[st_row]
            qtr = nb % w
            if ob_eng == "act":
                nc.scalar.copy(out=ob[:, qtr * SB:(qtr + 1) * SB], in_=op[:])
            else:
                nc.vector.tensor_copy(ob[:, qtr * SB:(qtr + 1) * SB], op[:])
            if nb % w == w - 1:
                rs = slice(st_row * 128, (st_row + 1) * 128)
                cs = slice((nb - w + 1) * SB, (nb + 1) * SB)
                nc.sync.dma_start(out=out[rs, cs], in_=ob[:, 0:w * SB])
                del ob_state[st_row]

        # -------------------------------------------------------------------
        # Round schedule:
        #   r0: proj0|rope0       r1: proj1|rope1|attn0
        #   r2: proj2|rope2|attn1|outproj0   r3: proj3|rope3|attn2|outproj1
        #   r4: attn3|outproj2    r5: outproj3
        # -------------------------------------------------------------------
        ACT_OB_SHARE = {2: 0.5, 3: 0.5, 4: 0.35, 5: 0.5}

        def emit_segment_b(r, drains, tails):
            ta = r - 1            # attention q-block this round
            to = r - 2            # out-projection q-block this round
            units = []            # PE-work units in emission order
            if 0 <= ta < N_SB:
                n = 4 * (ta + 1)
                for h in range(G):
                    units.append(("begin", ta, h))
                    for c in range(n):
                        units.append(("chunk", ta, h, c))
                    units.append(("eplg", ta, h))
            nop = 32 if 0 <= to < N_SB else 0
            nch = sum(1 for u in units if u[0] == "chunk")
            op_i = 0
            ob_flip = 0.0
            seq = []
            chunk_seen = 0
            nd, nt = len(drains), len(tails)
            if nch == 0:
                # no attention this round: emit drains/tails directly
                seq += [("drain", k) for k in range(nd)]
                seq += [("tail", k) for k in range(nt)]
                if r < N_SB:
                    seq.append(("transp", r))
            for u in units:
                if u[0] == "eplg":
                    # cover the Z-matmul's wait on the E-accumulators
                    for _ in range(2):
                        if op_i < nop:
                            seq.append(("op", to, op_i))
                            op_i += 1
                seq.append(u)
                if u[0] == "chunk":
                    chunk_seen += 1
                    # interleave PSUM drains / rope tails with early chunks,
                    # alternating so tail-i follows drain-i closely and the
                    # swap PSUM banks free early for the next projection
                    k = chunk_seen - 1
                    if k < 2 * min(nd, nt):
                        if k % 2 == 0:
                            seq.append(("drain", k // 2))
                        else:
                            seq.append(("tail", k // 2))
                    elif k < nd + nt:
                        seq.append(("drain", k - nt)
                                   if nd > nt else ("tail", k - nd))
                        if k == nd + nt - 1 and r < N_SB:
                            seq.append(("transp", r))
                    while nch and op_i < nop and (op_i + 1) / nop <= \
                            chunk_seen / nch:
                        seq.append(("op", to, op_i))
                        op_i += 1
            while op_i < nop:
                seq.append(("op", to, op_i))
                op_i += 1
            act_share = ACT_OB_SHARE.get(r, 0.0)
            for u in seq:
                kind = u[0]
                if kind == "begin":
                    attn_begin(u[1], u[2])
                elif kind == "chunk":
                    attn_chunk(u[1], u[2], u[3])
                elif kind == "eplg":
                    attn_eplg(u[1], u[2])
                elif kind == "transp":
                    emit_transposes(u[1])
                elif kind == "drain":
                    drains[u[1]]()
                elif kind == "tail":
                    tails[u[1]]()
                elif kind == "op":
                    ob_flip += act_share
                    if ob_flip >= 1.0:
                        ob_flip -= 1.0
                        eng = "act"
                    else:
                        eng = "dve"
                    outproj_tile(u[1], u[2], eng)

        emit_weight_dma()
        for r in range(N_SB + 2):
            drains, tails = [], []
            if r < N_SB:
                acc = emit_proj(r)
                drains = make_drain_units(r, acc)
                tails = make_tail_units(r)
            if r + 1 < N_SB:
                emit_x_dma(r + 1)
            if r == 1:
                for g in range(G):
                    nc.sync.dma_start(out=wo_r[:, g, :], in_=wot[:, g, :])
            emit_segment_b(r, drains, tails)

        if DEBUG_TAPS:
            for i in range(5):
                for sb in range(N_SB):
                    src = qr[i][sb] if i < G else kr[sb]
                    nc.sync.dma_start(out=dbg_qr[i, sb], in_=src[:])
            for sb in range(N_SB):
                nc.sync.dma_start(out=dbg_vn[sb], in_=vnat[sb][:])
            for h in range(G):
                for t in range(N_SB):
                    nc.sync.dma_start(out=dbg_at[h, t], in_=attn[h][t][:])

    nc.compile()
    return nc


def _prep_inputs(hidden_states, Wq, Wk, Wv, Wo, cos, sin):
    x = np.asarray(hidden_states, dtype=np.float32).reshape(S, D)
    Wq = np.asarray(Wq, dtype=np.float32)
    Wk = np.asarray(Wk, dtype=np.float32)
    Wv = np.asarray(Wv, dtype=np.float32)
    Wo = np.asarray(Wo, dtype=np.float32)
    cos = np.asarray(cos, dtype=np.float32)
    sin = np.asarray(sin, dtype=np.float32)

    # x pretiled: xg[sb, dcg, p, j, s] = x.T[dcg*512 + j*128 + p, sb*512 + s]
    xT = np.ascontiguousarray(x.T).astype(np.float16)
    xg = np.ascontiguousarray(
        xT.reshape(N_DCG, 4, 128, N_SB, SB).transpose(3, 0, 2, 1, 4))
    cosT = np.ascontiguousarray(cos.T).astype(np.float16)
    sinT = np.ascontiguousarray(sin.T).astype(np.float16)
    # lower-triangle (inclusive) 0/1 mask for the 128x128 diagonal block
    kp = np.arange(128)[:, None]
    qc = np.arange(128)[None, :]
    triT = (kp <= qc).astype(np.float16)
    # signed rotate-half permutation: out[i] = sum_p permT[p, i] * in[p]
    permT = np.zeros((128, 128), dtype=np.float16)
    half = Dh // 2
    for i in range(half):
        permT[i + half, i] = -1.0      # out[i<64] = -in[i+64]
    for i in range(half, Dh):
        permT[i - half, i] = 1.0       # out[i>=64] = +in[i-64]

    in_maps = []
    for c in range(N_CORES):
        wq_s = Wq[c * EH:(c + 1) * EH, :]          # [EH, D]
        wk_s = Wk[c * Dh:(c + 1) * Dh, :]
        wv_s = Wv[c * Dh:(c + 1) * Dh, :]
        wo_s = Wo[:, c * EH:(c + 1) * EH]          # [D, EH]
        # wqt[p, dc, e] = wq_s.T[dc*128+p, e]
        wqt = np.ascontiguousarray(
            np.ascontiguousarray(wq_s.T).astype(np.float16)
            .reshape(N_DC, 128, EH).transpose(1, 0, 2))
        wkt = np.ascontiguousarray(
            np.ascontiguousarray(wk_s.T).astype(np.float16)
            .reshape(N_DC, 128, Dh).transpose(1, 0, 2))
        wvt = np.ascontiguousarray(
            np.ascontiguousarray(wv_s.T).astype(np.float16)
            .reshape(N_DC, 128, Dh).transpose(1, 0, 2))
        # wot[p, h, d] = wo_s.T[h*128+p, d]
        wot = np.ascontiguousarray(
            np.ascontiguousarray(wo_s.T).astype(np.float16)
            .reshape(G, 128, D).transpose(1, 0, 2))
        in_maps.append({
            "xg": xg, "wqt": wqt, "wkt": wkt, "wvt": wvt, "wot": wot,
            "cosT": cosT, "sinT": sinT, "triT": triT, "permT": permT,
        })
    return in_maps


def run(trace=False, **inputs):
    """Run on hardware; returns (full_output, exec_time_ns or None)."""
    from concourse.bass_utils import run_bass_kernel_spmd

    if trace:
        _install_ntff_hook()
    if "nc" not in _cache:
        _cache["nc"] = _build()
    nc = _cache["nc"]
    in_maps = _prep_inputs(**inputs)
    res = run_bass_kernel_spmd(nc, in_maps, core_ids=list(range(N_CORES)),
                               trace=trace)
    acc = res.results[0]["out"].astype(np.float32)
    for c in range(1, N_CORES):
        acc += res.results[c]["out"]
    return acc.reshape(B, S, D), res.exec_time_ns


def _install_ntff_hook():
    """Register the axon NTFF profiling hook missing from this image."""
    import types
    try:
        import antenv
        from trn_agent_boot.trn_boot import _ntff_profile_via_ctypes
    except ImportError:
        return
    if "antenv.axon_hooks" in sys.modules:
        return
    mod = types.ModuleType("antenv.axon_hooks")
    mod._hook = _ntff_profile_via_ctypes("/opt/axon/libaxon_pjrt.so")
    mod.get_axon_ntff_profile_hook = lambda: mod._hook
    mod.set_axon_ntff_profile_hook = lambda h: setattr(mod, "_hook", h)
    sys.modules["antenv.axon_hooks"] = mod
    antenv.axon_hooks = mod


def kernel(**inputs):
    out, _ = run(trace=False, **inputs)
    return out


# revision 53
# speedup vs baseline: 1.0046x; 1.0046x over previous
"""GQA attention (BagleyAttention) on 8 Trainium2 NeuronCores.

Tensor-parallel over kv-head groups: core c owns kv head c and query heads
[4c, 4c+4). Each core computes its heads' attention and a partial output
projection [S, D]; the host sums the 8 partials.

v3: software-pipelined rounds — round r runs the sb=r QKV projection on the
PE while the previous round's attention (t=r-1) and the round-before's
output projection (t=r-2) fill the PE between exp-paced score chunks.
Softmax denominators accumulate in fp16 on DVE/GpSimd; causal diagonal
chunks stream only the valid q columns; rotate_half is a signed
permutation matmul (no SBUF-SBUF DMA); all HBM traffic uses host-pretiled
layouts so each DMA descriptor moves >=4KB per partition (the DMA fabric
is packet-rate-bound at ~1KB granularity).
"""

import math
import sys

sys.path.insert(0, "/opt/trn_rl_repo")

import numpy as np

# Problem sizes (hardcoded per contract; kernel.py reads no sibling files).
B, S, D = 1, 2048, 4096
H, KV, Dh = 32, 8, 128
G = H // KV            # query heads per kv head (= per core)
EH = G * Dh            # per-core q projection width (512)
N_CORES = 8

SB = 512               # s-block width (projection s-block = attention q-block)
N_SB = S // SB         # 4 rounds
N_DC = D // 128        # 32 d-chunks
N_DCG = 8              # d-chunk groups of 4 (x staging granularity)
N_NB = D // SB         # 8 output d-blocks

EXP_BIAS = 9.5         # exp(s - EXP_BIAS); cancels in softmax normalization

DEBUG_TAPS = False     # add qr/kr/vnat/attn dram dumps (debugging only)

_cache = {}


def _build():
    import concourse.bass as bass
    import concourse.mybir as mybir
    import concourse.tile as tile
    from concourse import bacc
    from concourse.masks import make_identity

    dt = mybir.dt
    f32, f16 = dt.float32, dt.float16
    AF = mybir.ActivationFunctionType

    nc = bacc.Bacc("TRN2", target_bir_lowering=False, debug=False)

    # host-pretiled inputs (see _prep_inputs for layouts)
    xg = nc.dram_tensor("xg", [N_SB, N_DCG, 128, 4, SB], f16,
                        kind="ExternalInput").ap()
    wqt = nc.dram_tensor("wqt", [128, N_DC, EH], f16,
                         kind="ExternalInput").ap()
    wkt = nc.dram_tensor("wkt", [128, N_DC, Dh], f16,
                         kind="ExternalInput").ap()
    wvt = nc.dram_tensor("wvt", [128, N_DC, Dh], f16,
                         kind="ExternalInput").ap()
    wot = nc.dram_tensor("wot", [128, G, D], f16, kind="ExternalInput").ap()
    cosT = nc.dram_tensor("cosT", [Dh, S], f16, kind="ExternalInput").ap()
    sinT = nc.dram_tensor("sinT", [Dh, S], f16, kind="ExternalInput").ap()
    triT = nc.dram_tensor("triT", [128, 128], f16, kind="ExternalInput").ap()
    permT = nc.dram_tensor("permT", [128, 128], f16,
                           kind="ExternalInput").ap()
    out = nc.dram_tensor("out", [S, D], f16, kind="ExternalOutput").ap()
    if DEBUG_TAPS:
        dbg_qr = nc.dram_tensor("dbg_qr", [5, N_SB, 128, SB], f16,
                                kind="ExternalOutput").ap()
        dbg_vn = nc.dram_tensor("dbg_vn", [N_SB, 128, N_SB * Dh], f16,
                                kind="ExternalOutput").ap()
        dbg_at = nc.dram_tensor("dbg_at", [G, N_SB, 128, SB], f16,
                                kind="ExternalOutput").ap()

    inv_sqrt_dh = 1.0 / math.sqrt(Dh)

    with tile.TileContext(nc) as tc, \
         tc.tile_pool(name="persist", bufs=1) as persist, \
         tc.tile_pool(name="mixp", bufs=6, space="PSUM") as mixp, \
         tc.tile_pool(name="scp", bufs=2, space="PSUM") as scp, \
         tc.tile_pool(name="xstage", bufs=9) as xstage, \
         tc.tile_pool(name="ropeb", bufs=1) as ropeb, \
         tc.tile_pool(name="expp", bufs=6) as expp, \
         tc.tile_pool(name="eaccp", bufs=2) as eaccp, \
         tc.tile_pool(name="miscp", bufs=2) as miscp, \
         tc.tile_pool(name="obuf", bufs=2) as obuf:

        # ---- long-lived SBUF tensors --------------------------------------
        qr = [[persist.tile([128, SB], f16, tag=f"qr{h}_{sb}",
                            name=f"qr{h}_{sb}") for sb in range(N_SB)]
              for h in range(G)]
        kr = [persist.tile([128, SB], f16, tag=f"kr{sb}", name=f"kr{sb}")
              for sb in range(N_SB)]
        vnat = [persist.tile([128, N_SB * Dh], f16, tag=f"vn{sb}",
                             name=f"vn{sb}") for sb in range(N_SB)]
        attn = [[persist.tile([128, SB], f16, tag=f"attn{h}_{t}",
                              name=f"attn{h}_{t}") for t in range(N_SB)]
                for h in range(G)]

        wq_h = persist.tile([128, N_DC, EH], f16, tag="wq_h", name="wq_h")
        wk_h = persist.tile([128, N_DC, Dh], f16, tag="wk_h", name="wk_h")
        wv_h = persist.tile([128, N_DC, Dh], f16, tag="wv_h", name="wv_h")
        wo_r = persist.tile([128, G, D], f16, tag="wo_r", name="wo_r")
        cos_sb = persist.tile([128, S], f16, tag="cos", name="cos_sb")
        sin_sb = persist.tile([128, S], f16, tag="sin", name="sin_sb")
        tri = persist.tile([128, 128], f16, tag="tri", name="tri")
        perm = persist.tile([128, 128], f16, tag="perm", name="perm")
        ones_h = persist.tile([128, 128], f16, tag="ones", name="ones_h")
        ident = persist.tile([128, 128], f16, tag="ident", name="ident")
        ebias = persist.tile([128, 1], f32, tag="ebias", name="ebias")

        nc.vector.memset(ones_h, 1.0)
        nc.vector.memset(ebias, -EXP_BIAS)
        make_identity(nc, ident)

        # PE warm-up: no-dep matmuls run during the initial DMA wait,
        # keeping the HAM activity window busy so the first projection
        # matmuls start at the full 2.4 GHz clock instead of 1.2 GHz.
        wsrc = persist.tile([128, SB], f16, tag="wsrc", name="wsrc")
        nc.vector.memset(wsrc, 0.000244140625)
        warm = mixp.tile([128, SB], f32, tag="mix", name="warm")
        for _ in range(28):
            nc.tensor.matmul(warm[:], ones_h[:], wsrc[:],
                             start=True, stop=True)

        # RoPE scratch (per projection head slot: 4 q + 1 k)
        t_plain = [ropeb.tile([128, SB], f16, tag=f"plain{i}",
                              name=f"plain{i}") for i in range(5)]
        t_cos = [ropeb.tile([128, SB], f16, tag=f"tcos{i}",
                            name=f"tcos{i}") for i in range(5)]
        t_sw = [ropeb.tile([128, SB], f16, tag=f"tsw{i}",
                           name=f"tsw{i}") for i in range(5)]
        vt_sb = ropeb.tile([128, SB], f16, tag="vt", name="vt_sb")

        # -------------------------------------------------------------------
        # DMA emission (all host-pretiled, big per-partition rows)
        # -------------------------------------------------------------------
        xq_tiles = {}   # (sb, dcg) -> tile [128, 4, SB]

        def emit_x_dma(sb):
            for dcg in range(N_DCG):
                xq = xstage.tile([128, 4, SB], f16, tag="xq",
                                 name=f"xq{sb}_{dcg}")
                nc.sync.dma_start(out=xq, in_=xg[sb, dcg])
                xq_tiles[(sb, dcg)] = xq

        def emit_weight_dma():
            # order: first dc-group of everything, then interleave so DMA
            # supply stays ~2 dc-groups ahead of the PE during round 0.
            def wq_g(g):
                cs = slice(g * 8, (g + 1) * 8)
                nc.sync.dma_start(out=wq_h[:, cs, :], in_=wqt[:, cs, :])

            def wkv_g(g):
                cs = slice(g * 16, (g + 1) * 16)
                nc.sync.dma_start(out=wk_h[:, cs, :], in_=wkt[:, cs, :])
                nc.sync.dma_start(out=wv_h[:, cs, :], in_=wvt[:, cs, :])

            def xq_g(dcg):
                xq = xstage.tile([128, 4, SB], f16, tag="xq",
                                 name=f"xq0_{dcg}")
                nc.sync.dma_start(out=xq, in_=xg[0, dcg])
                xq_tiles[(0, dcg)] = xq

            # small first descriptors so the first matmul starts ASAP
            nc.sync.dma_start(out=wq_h[:, 0:2, :], in_=wqt[:, 0:2, :])
            nc.sync.dma_start(out=wk_h[:, 0:4, :], in_=wkt[:, 0:4, :])
            nc.sync.dma_start(out=wv_h[:, 0:4, :], in_=wvt[:, 0:4, :])
            xq_g(0)
            nc.sync.dma_start(out=wq_h[:, 2:8, :], in_=wqt[:, 2:8, :])
            nc.sync.dma_start(out=wk_h[:, 4:8, :], in_=wkt[:, 4:8, :])
            nc.sync.dma_start(out=wv_h[:, 4:8, :], in_=wvt[:, 4:8, :])
            xq_g(1)
            nc.sync.dma_start(out=wk_h[:, 8:16, :], in_=wkt[:, 8:16, :])
            nc.sync.dma_start(out=wv_h[:, 8:16, :], in_=wvt[:, 8:16, :])
            wq_g(1); xq_g(2); xq_g(3)
            wq_g(2); wkv_g(1); xq_g(4); xq_g(5)
            wq_g(3); xq_g(6); xq_g(7)
            # trig/masks land before their first readers (rope drain 0 /
            # rope tail 0 / first diagonal chunk); must be EMITTED before
            # those readers too, else Tile orders the load after them.
            nc.sync.dma_start(out=cos_sb, in_=cosT)
            nc.sync.dma_start(out=sin_sb, in_=sinT)
            nc.sync.dma_start(out=tri, in_=triT)
            nc.sync.dma_start(out=perm, in_=permT)

        # -------------------------------------------------------------------
        # Projection: 6 accumulation groups (4 q heads, k, v) over 32 d-chunks
        # -------------------------------------------------------------------
        def emit_proj(r):
            acc = [mixp.tile([128, SB], f32, tag="mix", name=f"acc{r}_{i}")
                   for i in range(6)]
            for dcg in range(N_DCG):
                xf = xq_tiles.pop((r, dcg))
                for j in range(4):
                    dc = dcg * 4 + j
                    xs = xf[:, j, :]
                    st_flags = dict(start=(dc == 0), stop=(dc == N_DC - 1))
                    for h in range(G):
                        nc.tensor.matmul(
                            acc[h][:], wq_h[:, dc, h * 128:(h + 1) * 128],
                            xs, **st_flags)
                    nc.tensor.matmul(acc[4][:], wk_h[:, dc, :], xs,
                                     **st_flags)
                    nc.tensor.matmul(acc[5][:], wv_h[:, dc, :], xs,
                                     **st_flags)
            return acc

        # RoPE part 1: drain the 6 PSUM banks (plain copies on ACT; cos-muls
        # on DVE). Frees "mix" banks for segment B; interleaved there.
        def make_drain_units(r, acc):
            ss = slice(r * SB, (r + 1) * SB)

            def drain(i):
                if i < 5:
                    nc.scalar.copy(out=t_plain[i], in_=acc[i][:])
                    nc.vector.tensor_mul(t_cos[i], acc[i][:], cos_sb[:, ss])
                else:
                    nc.scalar.copy(out=vt_sb, in_=acc[5][:])
            return [lambda i=i: drain(i) for i in range(6)]

        # RoPE part 2: rotate-half via signed permutation matmul, then
        # combine.  swp = P @ plain; dst = cos*acc + sin*swp.
        def make_tail_units(r):
            ss = slice(r * SB, (r + 1) * SB)

            def tail(i):
                swp = mixp.tile([128, SB], f32, tag="mix", name=f"swp{r}_{i}")
                nc.tensor.matmul(swp[:], perm[:], t_plain[i][:],
                                 start=True, stop=True)
                nc.vector.tensor_mul(t_sw[i], swp[:], sin_sb[:, ss])
                dst = qr[i][r] if i < G else kr[r]
                # final combine on GpSimd (SBUF-only op; keeps DVE free)
                nc.gpsimd.tensor_add(dst[:], t_cos[i], t_sw[i])
            return [lambda i=i: tail(i) for i in range(5)]

        # V natural layout: 4 PE transposes into one PSUM bank, one copy out.
        def emit_transposes(r):
            tp = mixp.tile([128, SB], f16, tag="mix", name=f"tp{r}")
            for j in range(4):
                nc.tensor.transpose(tp[:, j * 128:(j + 1) * 128],
                                    vt_sb[:, j * 128:(j + 1) * 128], ident[:])
            nc.vector.tensor_copy(vnat[r][:], tp[:])

        # -------------------------------------------------------------------
        # Attention for q-block t, head h (chunk-at-a-time emission)
        # -------------------------------------------------------------------
        att_state = {}

        def attn_begin(t, h):
            n = 4 * (t + 1)
            st = dict(n=n, pv=mixp.tile([128, SB], f32, tag="mix",
                                        name=f"pv{t}_{h}"))
            st["ed"] = eaccp.tile([128, SB], f16, tag="ed", name="ed")
            st["eg"] = eaccp.tile([128, SB], f16, tag="eg", name="eg")
            att_state[(t, h)] = st
            return st

        def attn_chunk(t, h, c):
            st = att_state[(t, h)]
            qlo = 128 * (c - 4 * t) if c >= 4 * t else 0
            W = SB - qlo
            sc = scp.tile([128, SB], f32, tag="sc", name="sc")
            nc.tensor.matmul(sc[:, 0:W], kr[c // 4][:, (c % 4) * 128:
                                                    (c % 4) * 128 + 128],
                             qr[h][t][:, qlo:SB], start=True, stop=True)
            e = expp.tile([128, SB], f16, tag="e", name="e")
            nc.scalar.activation(e[:, 0:W], sc[:, 0:W], AF.Exp,
                                 scale=inv_sqrt_dh, bias=ebias[:])
            # diagonal chunk: mask first 128 q-cols (cheap [128,128] on DVE;
            # this is on the pv-matmul's critical path — keep it fast)
            if c >= 4 * t:
                nc.vector.tensor_mul(e[:, 0:128], e[:, 0:128], tri[:])
            # E-accumulator: DVE owns even chunks plus the final (odd)
            # chunk — the Z-matmul should not wait on slower GpSimd adds.
            on_dve = (c % 2 == 0) or (c == st["n"] - 1)
            eng = nc.vector if on_dve else nc.gpsimd
            dst = st["ed"] if on_dve else st["eg"]
            if c == 1 and t == 0:
                # odd accumulator's first chunk only covers q >= 128
                nc.gpsimd.memset(st["eg"][:, 0:128], 0.0)
            if c <= 1:
                eng.tensor_copy(dst[:, qlo:SB], e[:, 0:W])
            else:
                eng.tensor_add(dst[:, qlo:SB], dst[:, qlo:SB], e[:, 0:W])
            vw = vnat[c // 4][:, (c % 4) * Dh:(c % 4 + 1) * Dh]
            nc.tensor.matmul(st["pv"][:, qlo:SB], vw, e[:, 0:W],
                             start=(c == 0), stop=(c == st["n"] - 1))

        def attn_eplg(t, h):
            st = att_state.pop((t, h))
            zb = mixp.tile([128, SB], f32, tag="mix", name=f"z{t}_{h}")
            nc.tensor.matmul(zb[:], ones_h[:], st["ed"][:],
                             start=True, stop=False)
            nc.tensor.matmul(zb[:], ones_h[:], st["eg"][:],
                             start=False, stop=True)
            rz = miscp.tile([128, SB], f32, tag="rz", name="rz")
            nc.vector.reciprocal_approx_fast(out=rz, in_=zb[:])
            nc.vector.tensor_mul(attn[h][t][:], st["pv"][:], rz[:])

        # -------------------------------------------------------------------
        # Output projection: 8 groups of 4 d-blocks per q-block; one DMA
        # per group ([128, 2048] = 4KB rows).
        # -------------------------------------------------------------------
        ob_state = {}

        def outproj_tile(t, i, ob_eng):
            sl = (i // N_NB) * 128          # s-offset within block
            nb = i % N_NB
            st_row = 4 * t + i // N_NB
            op = mixp.tile([128, SB], f32, tag="mix", name=f"op{t}_{i}")
            for hh in range(G):
                nc.tensor.matmul(op[:], attn[hh][t][:, sl:sl + 128],
                                 wo_r[:, hh, nb * SB:(nb + 1) * SB],
                                 start=(hh == 0), stop=(hh == G - 1))
            # group width: 4 d-blocks per DMA; 2 for the very last s-tile so
            # the final transfer starts earlier (shorter kernel tail)
            w = 2 if (t == 3 and i >= 24) else 4
            if nb % w == 0:
                ob_state[st_row] = obuf.tile([128, w * SB], f16, tag="ob",
                                             name=f"ob{t}_{i}")
            ob = ob_state[st_row]
            qtr = nb % w
            if ob_eng == "act":
                nc.scalar.copy(out=ob[:, qtr * SB:(qtr + 1) * SB], in_=op[:])
            else:
                nc.vector.tensor_copy(ob[:, qtr * SB:(qtr + 1) * SB], op[:])
            if nb % w == w - 1:
                rs = slice(st_row * 128, (st_row + 1) * 128)
                cs = slice((nb - w + 1) * SB, (nb + 1) * SB)
                nc.sync.dma_start(out=out[rs, cs], in_=ob[:, 0:w * SB])
                del ob_state[st_row]

        # -------------------------------------------------------------------
        # Round schedule:
        #   r0: proj0|rope0       r1: proj1|rope1|attn0
        #   r2: proj2|rope2|attn1|outproj0   r3: proj3|rope3|attn2|outproj1
        #   r4: attn3|outproj2    r5: outproj3
        # -------------------------------------------------------------------
        ACT_OB_SHARE = {2: 0.5, 3: 0.5, 4: 0.35, 5: 0.5}

        def emit_segment_b(r, drains, tails):
            ta = r - 1            # attention q-block this round
            to = r - 2            # out-projection q-block this round
            units = []            # PE-work units in emission order
            if 0 <= ta < N_SB:
                n = 4 * (ta + 1)
                for h in range(G):
                    units.append(("begin", ta, h))
                    for c in range(n):
                        units.append(("chunk", ta, h, c))
                    units.append(("eplg", ta, h))
            nop = 32 if 0 <= to < N_SB else 0
            nch = sum(1 for u in units if u[0] == "chunk")
            op_i = 0
            ob_flip = 0.0
            seq = []
            chunk_seen = 0
            nd, nt = len(drains), len(tails)
            if nch == 0:
                # no attention this round: emit drains/tails directly
                seq += [("drain", k) for k in range(nd)]
                seq += [("tail", k) for k in range(nt)]
                if r < N_SB:
                    seq.append(("transp", r))
            for u in units:
                if u[0] == "eplg":
                    # cover the Z-matmul's wait on the E-accumulators
                    for _ in range(2):
                        if op_i < nop:
                            seq.append(("op", to, op_i))
                            op_i += 1
                seq.append(u)
                if u[0] == "chunk":
                    chunk_seen += 1
                    # interleave PSUM drains / rope tails with early chunks,
                    # alternating so tail-i follows drain-i closely and the
                    # swap PSUM banks free early for the next projection
                    k = chunk_seen - 1
                    if k < 2 * min(nd, nt):
                        if k % 2 == 0:
                            seq.append(("drain", k // 2))
                        else:
                            seq.append(("tail", k // 2))
                    elif k < nd + nt:
                        seq.append(("drain", k - nt)
                                   if nd > nt else ("tail", k - nd))
                        if k == nd + nt - 1 and r < N_SB:
                            seq.append(("transp", r))
                    while nch and op_i < nop and (op_i + 1) / nop <= \
                            chunk_seen / nch:
                        seq.append(("op", to, op_i))
                        op_i += 1
            while op_i < nop:
                seq.append(("op", to, op_i))
                op_i += 1
            act_share = ACT_OB_SHARE.get(r, 0.0)
            for u in seq:
                kind = u[0]
                if kind == "begin":
                    attn_begin(u[1], u[2])
                elif kind == "chunk":
                    attn_chunk(u[1], u[2], u[3])
                elif kind == "eplg":
                    attn_eplg(u[1], u[2])
                elif kind == "transp":
                    emit_transposes(u[1])
                elif kind == "drain":
                    drains[u[1]]()
                elif kind == "tail":
                    tails[u[1]]()
                elif kind == "op":
                    ob_flip += act_share
                    if ob_flip >= 1.0:
                        ob_flip -= 1.0
                        eng = "act"
                    else:
                        eng = "dve"
                    outproj_tile(u[1], u[2], eng)

        emit_weight_dma()
        for r in range(N_SB + 2):
            drains, tails = [], []
            if r < N_SB:
                acc = emit_proj(r)
                drains = make_drain_units(r, acc)
                tails = make_tail_units(r)
            if r + 1 < N_SB:
                emit_x_dma(r + 1)
            if r == 1:
                for g in range(G):
                    nc.sync.dma_start(out=wo_r[:, g, :], in_=wot[:, g, :])
            emit_segment_b(r, drains, tails)

        if DEBUG_TAPS:
            for i in range(5):
                for sb in range(N_SB):
                    src = qr[i][sb] if i < G else kr[sb]
                    nc.sync.dma_start(out=dbg_qr[i, sb], in_=src[:])
            for sb in range(N_SB):
                nc.sync.dma_start(out=dbg_vn[sb], in_=vnat[sb][:])
            for h in range(G):
                for t in range(N_SB):
                    nc.sync.dma_start(out=dbg_at[h, t], in_=attn[h][t][:])

    nc.compile()
    return nc


def _prep_inputs(hidden_states, Wq, Wk, Wv, Wo, cos, sin):
    x = np.asarray(hidden_states, dtype=np.float32).reshape(S, D)
    Wq = np.asarray(Wq, dtype=np.float32)
    Wk = np.asarray(Wk, dtype=np.float32)
    Wv = np.asarray(Wv, dtype=np.float32)
    Wo = np.asarray(Wo, dtype=np.float32)
    cos = np.asarray(cos, dtype=np.float32)
    sin = np.asarray(sin, dtype=np.float32)

    # x pretiled: xg[sb, dcg, p, j, s] = x.T[dcg*512 + j*128 + p, sb*512 + s]
    xT = np.ascontiguousarray(x.T).astype(np.float16)
    xg = np.ascontiguousarray(
        xT.reshape(N_DCG, 4, 128, N_SB, SB).transpose(3, 0, 2, 1, 4))
    cosT = np.ascontiguousarray(cos.T).astype(np.float16)
    sinT = np.ascontiguousarray(sin.T).astype(np.float16)
    # lower-triangle (inclusive) 0/1 mask for the 128x128 diagonal block
    kp = np.arange(128)[:, None]
    qc = np.arange(128)[None, :]
    triT = (kp <= qc).astype(np.float16)
    # signed rotate-half permutation: out[i] = sum_p permT[p, i] * in[p]
    permT = np.zeros((128, 128), dtype=np.float16)
    half = Dh // 2
    for i in range(half):
        permT[i + half, i] = -1.0      # out[i<64] = -in[i+64]
    for i in range(half, Dh):
        permT[i - half, i] = 1.0       # out[i>=64] = +in[i-64]

    in_maps = []
    for c in range(N_CORES):
        wq_s = Wq[c * EH:(c + 1) * EH, :]          # [EH, D]
        wk_s = Wk[c * Dh:(c + 1) * Dh, :]
        wv_s = Wv[c * Dh:(c + 1) * Dh, :]
        wo_s = Wo[:, c * EH:(c + 1) * EH]          # [D, EH]
        # wqt[p, dc, e] = wq_s.T[dc*128+p, e]
        wqt = np.ascontiguousarray(
            np.ascontiguousarray(wq_s.T).astype(np.float16)
            .reshape(N_DC, 128, EH).transpose(1, 0, 2))
        wkt = np.ascontiguousarray(
            np.ascontiguousarray(wk_s.T).astype(np.float16)
            .reshape(N_DC, 128, Dh).transpose(1, 0, 2))
        wvt = np.ascontiguousarray(
            np.ascontiguousarray(wv_s.T).astype(np.float16)
            .reshape(N_DC, 128, Dh).transpose(1, 0, 2))
        # wot[p, h, d] = wo_s.T[h*128+p, d]
        wot = np.ascontiguousarray(
            np.ascontiguousarray(wo_s.T).astype(np.float16)
            .reshape(G, 128, D).transpose(1, 0, 2))
        in_maps.append({
            "xg": xg, "wqt": wqt, "wkt": wkt, "wvt": wvt, "wot": wot,
            "cosT": cosT, "sinT": sinT, "triT": triT, "permT": permT,
        })
    return in_maps


def run(trace=False, **inputs):
    """Run on hardware; returns (full_output, exec_time_ns or None)."""
    from concourse.bass_utils import run_bass_kernel_spmd

    if trace:
        _install_ntff_hook()
    if "nc" not in _cache:
        _cache["nc"] = _build()
    nc = _cache["nc"]
    in_maps = _prep_inputs(**inputs)
    res = run_bass_kernel_spmd(nc, in_maps, core_ids=list(range(N_CORES)),
                               trace=trace)
    acc = res.results[0]["out"].astype(np.float32)
    for c in range(1, N_CORES):
        acc += res.results[c]["out"]
    return acc.reshape(B, S, D), res.exec_time_ns


def _install_ntff_hook():
    """Register the axon NTFF profiling hook missing from this image."""
    import types
    try:
        import antenv
        from trn_agent_boot.trn_boot import _ntff_profile_via_ctypes
    except ImportError:
        return
    if "antenv.axon_hooks" in sys.modules:
        return
    mod = types.ModuleType("antenv.axon_hooks")
    mod._hook = _ntff_profile_via_ctypes("/opt/axon/libaxon_pjrt.so")
    mod.get_axon_ntff_profile_hook = lambda: mod._hook
    mod.set_axon_ntff_profile_hook = lambda h: setattr(mod, "_hook", h)
    sys.modules["antenv.axon_hooks"] = mod
    antenv.axon_hooks = mod


def kernel(**inputs):
    out, _ = run(trace=False, **inputs)
    return out
